# revision 1
# baseline (speedup 1.0000x reference)
"""Masked multi-head attention (B=2, H=16, S=2048, D=64) on 8 TRN2 NeuronCores.

Sharding: batch*heads (32) split 4-heads-per-core across 8 cores; each core
computes full attention for its heads; the boolean mask is shared (broadcast
to every core). No cross-device communication.

Device algorithm (per head), computed in transposed "S^T" layout so the
softmax probabilities land with the contraction (k) dim on partitions and
feed the P@V matmul with no on-device transposes:

  S^T[k, q]  = K^T[d, k].T @ Q^T[d, q]          (PE, d=64 contraction,
                                                 consecutive k-blocks packed
                                                 in opposite PE row halves)
  P^T[k, q]  = exp(scale * S^T) * keepT[k, q]   (ACT exp from PSUM; keep-mult
                                                 alternating DVE/GPSIMD, one
                                                 op per k-block pair)
  O^T_aug    = V_aug[k, d+1].T @ P^T[k, q]      (PE, accumulated over k in
                                                 PSUM; V_aug has a ones
                                                 column -> row d holds the
                                                 softmax denominators)

O^T_aug [65, 2048] fp32 is copied PSUM->SBUF (DVE) and DMA'd to HBM; the
final normalize (divide by denominator row) + un-transpose happens on host.

Softmax needs no running-max: scores*scale ~ N(0,1), |max| < ~7, exp() is
safely in fp32 range, and softmax is shift-invariant so the result matches
the reference's max-subtracted computation. Masked entries match exactly:
reference adds -1e4 so exp underflows to 0.0 in fp32; we multiply by 0.
"""

import sys

for _p in ("/opt/trn_rl_repo", "/root/.axon_site/_ro/trn_rl_repo"):
    if _p not in sys.path:
        sys.path.append(_p)

import numpy as np
import ml_dtypes

B, H, S, D = 2, 16, 2048, 64
N_CORES = 8
HPC = (B * H) // N_CORES  # heads per core
P = 128
KB = S // P               # k blocks per head
QH = 2                    # q halves (exp tile free dim = S/QH)
QW = S // QH
SCALE = 1.0 / 8.0         # 1/sqrt(D)

QK_DTYPE = "float32r"     # "bfloat16" | "float32r" (QK matmul operand dtype)
DEFAULT_OPTS = ("fp16",)  # p/V/keep in fp16: same speed as bf16, ~7x accuracy

_CACHE = {}


def _build(repeats=1, qk_dtype=QK_DTYPE, opts=()):
    opts = frozenset(opts)
    import concourse.mybir as mybir
    import concourse.tile as tile
    from concourse import bacc

    dt = mybir.dt
    qk_dt = getattr(dt, qk_dtype)
    half_dt = dt.float16 if "fp16" in opts else dt.bfloat16
    nc = bacc.Bacc(
        "TRN2", target_bir_lowering=False, debug=False, num_devices=N_CORES
    )

    qT = nc.dram_tensor("qT", [HPC, D, S], qk_dt, kind="ExternalInput").ap()
    kT = nc.dram_tensor("kT", [HPC, D, S], qk_dt, kind="ExternalInput").ap()
    v = nc.dram_tensor("v", [HPC, S, D], half_dt, kind="ExternalInput").ap()
    keepT = nc.dram_tensor("keepT", [S, S], half_dt, kind="ExternalInput").ap()
    out = nc.dram_tensor(
        "out", [HPC, D + 1, S], dt.float32, kind="ExternalOutput"
    ).ap()

    Exp = mybir.ActivationFunctionType.Exp
    mult = mybir.AluOpType.mult
    add = mybir.AluOpType.add
    band = mybir.AluOpType.bitwise_and
    bor = mybir.AluOpType.bitwise_or

    # Schraudolph exp2-bitcast constants for the DVE-exp offload path:
    # exp(SCALE*s) ~= bitcast_f32(int32(EA*s + EB)) * g(mantissa), with the
    # sawtooth corrected by a quadratic in the mantissa m in [1,2):
    # g(m) ~= XC2*m^2 + XC1*m + XC0 (rel-err fit of 2^(m-1)/m, max 6.6e-3).
    EA = float(2.0**23 * SCALE / np.log(2.0))
    EB = float(127 * 2**23)
    XC2, XC1, XC0 = 0.2256645362713346, -0.6662353885565159, 1.4340054811521055

    with tile.TileContext(nc) as tc:
        with (
            tc.tile_pool(name="keep_pool", bufs=1) as keep_pool,
            tc.tile_pool(name="qk_pool", bufs=(3 if "pipe" in opts else 2)) as qk_pool,
            tc.tile_pool(name="v_pool", bufs=2) as v_pool,
            tc.tile_pool(name="p_pool", bufs=(4 if "p4" in opts else 3)) as p_pool,
            tc.tile_pool(name="ob_pool", bufs=(3 if "pipe" in opts else 2)) as ob_pool,
            tc.tile_pool(name="x_pool", bufs=1) as x_pool,
            tc.tile_pool(
                name="s_psum",
                bufs=(2 if "s2o2" in opts else 3),
                space="PSUM",
            ) as s_psum,
            tc.tile_pool(
                name="o_psum",
                bufs=(2 if "s2o2" in opts else 1),
                space="PSUM",
            ) as o_psum,
        ):
            keep_sb = keep_pool.tile([P, KB, S], half_dt)
            for kb in range(KB):
                nc.sync.dma_start(
                    out=keep_sb[:, kb, :], in_=keepT[kb * P:(kb + 1) * P, :]
                )

            def body(rep):
                for h in range(HPC):
                    # Q^T/K^T [64, S] duplicated into both partition halves
                    # so consecutive k-blocks run in opposite PE row halves
                    # (tile_position packing via base_partition).
                    qTr = qk_pool.tile([P, S], qk_dt, tag="qTr", name=f"qTr_{h}")
                    kTr = qk_pool.tile([P, S], qk_dt, tag="kTr", name=f"kTr_{h}")
                    for half in (0, 1):
                        nc.sync.dma_start(
                            out=qTr[half * 64:(half + 1) * 64, :], in_=qT[h]
                        )
                        nc.sync.dma_start(
                            out=kTr[half * 64:(half + 1) * 64, :], in_=kT[h]
                        )

                    # V with an appended ones column (denominator column).
                    v_sb = v_pool.tile(
                        [P, KB, D + 1], half_dt, tag="v", name=f"v_{h}"
                    )
                    v_re = v[h].rearrange("(kb p) d -> p kb d", p=P)
                    if "pipe" in opts:
                        nc.sync.dma_start(out=v_sb[:, 0:KB // 2, 0:D],
                                          in_=v_re[:, 0:KB // 2, :])
                        nc.sync.dma_start(out=v_sb[:, KB // 2:KB, 0:D],
                                          in_=v_re[:, KB // 2:KB, :])
                    else:
                        nc.sync.dma_start(out=v_sb[:, :, 0:D], in_=v_re)
                    nc.gpsimd.memset(v_sb[:, :, D:D + 1], 1.0)

                    for qh in range(QH):
                        ot = o_psum.tile(
                            [D + 1, QW], dt.float32, tag="ot", name=f"ot_{h}_{qh}"
                        )
                        for kbp in range(KB // 2):
                            use_dve = "dvexp" in opts and kbp == 5
                            if use_dve:
                                xi = x_pool.tile([P, 2, QW], dt.int32,
                                                 tag="xi", name=f"xi_{h}_{qh}")
                                xm = x_pool.tile([P, 2, QW], dt.int32,
                                                 tag="xm", name=f"xm_{h}_{qh}")
                                xh1 = x_pool.tile([P, 2, QW], dt.float32,
                                                  tag="xh1", name=f"xh1_{h}_{qh}")
                                xh2 = x_pool.tile([P, 2, QW], dt.float32,
                                                  tag="xh2", name=f"xh2_{h}_{qh}")
                                xy = x_pool.tile([P, 2, QW], dt.float32,
                                                 tag="xy", name=f"xy_{h}_{qh}")
                            else:
                                p2 = p_pool.tile(
                                    [P, 2, QW], half_dt, tag="p",
                                    name=f"p_{h}_{qh}_{kbp}",
                                )
                            for e in (0, 1):
                                kb = 2 * kbp + e
                                half = 64 * e
                                s_ps = s_psum.tile(
                                    [P, QW], dt.float32, tag="s",
                                    name=f"s_{h}_{qh}_{kb}",
                                )
                                import contextlib
                                prio = (
                                    tc.high_priority(offset=48)
                                    if "hiqk" in opts else contextlib.nullcontext()
                                )
                                with prio:
                                    for qc in range(QW // 512):
                                        q0 = qh * QW + qc * 512
                                        nc.tensor.matmul(
                                            s_ps[:, qc * 512:(qc + 1) * 512],
                                            lhsT=kTr[half:half + 64, kb * P:(kb + 1) * P],
                                            rhs=qTr[half:half + 64, q0:q0 + 512],
                                            start=True,
                                            stop=True,
                                        )
                                if use_dve:
                                    m = xm[:, e, :].bitcast(dt.float32)
                                    y0 = xi[:, e, :].bitcast(dt.float32)
                                    nc.vector.tensor_scalar(
                                        xi[:, e, :], s_ps[:, :], EA, EB, mult, add
                                    )
                                    nc.vector.tensor_scalar(
                                        xm[:, e, :], xi[:, e, :],
                                        0x007FFFFF, 0x3F800000, band, bor,
                                    )
                                    nc.vector.tensor_scalar(
                                        xh1[:, e, :], m, XC2, XC1, mult, add
                                    )
                                    nc.vector.tensor_tensor(
                                        xh2[:, e, :], xh1[:, e, :], m, mult
                                    )
                                    nc.vector.scalar_tensor_tensor(
                                        xy[:, e, :], xh2[:, e, :], XC0, y0,
                                        add, mult,
                                    )
                                else:
                                    nc.scalar.activation(
                                        p2[:, e, :], s_ps[:, :], Exp, scale=SCALE
                                    )
                            # Masking: one DVE multiply per k-block pair.
                            pm2 = p_pool.tile(
                                [P, 2, QW], half_dt, tag="pm",
                                name=f"pm_{h}_{qh}_{kbp}",
                            )
                            nc.vector.tensor_tensor(
                                pm2[:, :, :],
                                xy[:, :, :] if use_dve else p2[:, :, :],
                                keep_sb[:, 2 * kbp:2 * kbp + 2,
                                        qh * QW:(qh + 1) * QW],
                                mult,
                            )
                            for e in (0, 1):
                                kb = 2 * kbp + e
                                for qc in range(QW // 512):
                                    nc.tensor.matmul(
                                        ot[:, qc * 512:(qc + 1) * 512],
                                        lhsT=v_sb[:, kb, :],
                                        rhs=pm2[:, e, qc * 512:(qc + 1) * 512],
                                        start=(kb == 0),
                                        stop=(kb == KB - 1),
                                    )

                        # Output copy on the (under-used) scalar engine so the
                        # PSUM accumulator frees without stalling PE/DVE.
                        ob_sb = ob_pool.tile(
                            [D + 1, QW], dt.float32, tag="ob", name=f"ob_{h}_{qh}"
                        )
                        if "ob_dve" in opts or ("obsplit" in opts and qh == 1):
                            nc.vector.tensor_copy(ob_sb[:, :], ot[:, :])
                        else:
                            nc.scalar.copy(ob_sb[:, :], ot[:, :])
                        nc.sync.dma_start(
                            out=out[h][:, qh * QW:(qh + 1) * QW], in_=ob_sb[:, :]
                        )

            if repeats == 1:
                body(0)
            else:
                with tc.For_i(
                    0, repeats, 1,
                    hint_engines=(mybir.EngineType.PE, mybir.EngineType.DVE),
                ):
                    body(0)

    nc.compile()
    return nc


def get_nc(repeats=1, qk_dtype=QK_DTYPE, opts=()):
    key = ("nc", repeats, qk_dtype, frozenset(opts))
    if key not in _CACHE:
        _CACHE[key] = _build(repeats, qk_dtype, opts)
    return _CACHE[key]


def prep_in_maps(q, k, v, mask, qk_dtype=QK_DTYPE, half="float16"):
    bf16 = np.float16 if half == "float16" else ml_dtypes.bfloat16
    qk_np = np.float32 if qk_dtype == "float32r" else bf16
    q = np.asarray(q, dtype=np.float32).reshape(B * H, S, D)
    k = np.asarray(k, dtype=np.float32).reshape(B * H, S, D)
    vv = np.asarray(v, dtype=np.float32).reshape(B * H, S, D)
    mask = np.asarray(mask).reshape(S, S)
    keepT = np.ascontiguousarray((1 - mask).T.astype(np.float32)).astype(bf16)
    in_maps = []
    for c in range(N_CORES):
        sl = slice(c * HPC, (c + 1) * HPC)
        in_maps.append({
            "qT": np.ascontiguousarray(q[sl].transpose(0, 2, 1)).astype(qk_np),
            "kT": np.ascontiguousarray(k[sl].transpose(0, 2, 1)).astype(qk_np),
            "v": vv[sl].astype(bf16),
            "keepT": keepT,
        })
    return in_maps


def finish_output(core_results):
    """core_results: list of [HPC, D+1, S] fp32 arrays -> [B, H, S, D] fp32."""
    outs = []
    for r in core_results:
        r = np.asarray(r, dtype=np.float32)
        o = (r[:, :D, :] / r[:, D:D + 1, :]).transpose(0, 2, 1)
        outs.append(o)
    return np.concatenate(outs, axis=0).reshape(B, H, S, D).astype(np.float32)


def kernel(q, k, v, mask):
    from concourse import bass_utils

    nc = get_nc(1, opts=DEFAULT_OPTS)
    in_maps = prep_in_maps(q, k, v, mask)
    bkr = bass_utils.run_bass_kernel_spmd(nc, in_maps, list(range(N_CORES)))
    return finish_output([bkr.results[c]["out"] for c in range(N_CORES)])

